# revision 1
# baseline (speedup 1.0000x reference)
"""2-layer GATv2 (N=50000, E=1.6M, D=H=128, O=64) on 8 trn2 NeuronCores.

Strategy: 1D partition by destination node (each core owns 6272 dst nodes,
edges live with their dst owner). Per 128-edge tile: source features are
fetched by indirect DMA from a replicated xl table; destination features are
expanded on the tensor engine with an on-chip-built one-hot; attention
softmax is computed unnormalized (exp without max-subtraction is safe at
these scales) with the segment sum obtained by appending a ones-column to
the aggregation matmul; normalization is a per-node reciprocal after PSUM
accumulation. Layer-2 source table (xl2) is AllGather'ed between layers.
All biases are folded host-side (bl into the output constant since
sum(alpha)=1; bl+br into xr; blin+bias into the skip constant).
"""
import json
import sys
import types

import ml_dtypes
import numpy as np

import concourse.bass as bass
import concourse.mybir as mybir
import concourse.tile as tile
from concourse.masks import make_identity
from concourse.vector_clock import ScopedClock

F32 = mybir.dt.float32
BF16 = mybir.dt.bfloat16
I32 = mybir.dt.int32
AL = mybir.AluOpType
ACTF = mybir.ActivationFunctionType
NEG = 0.2

# ---------------------------------------------------------------------------
# environment fixups (walrus single-sync-wait limit)
# ---------------------------------------------------------------------------
_SPLIT_SEQ = [0]


def _split_multi_waits_json(m):
    for fn in m.get("functions", []):
        for bb in fn.get("blocks", []):
            insts = bb.get("instructions")
            if not insts:
                continue
            out = []
            for inst in insts:
                si = inst.get("sync_info")
                waits = si.get("on_wait") if si else None
                if waits and len(waits) > 1:
                    for w in waits[:-1]:
                        _SPLIT_SEQ[0] += 1
                        out.append({
                            "debug": inst.get("debug", 0),
                            "engine": inst["engine"],
                            "ins": [], "outs": [],
                            "name": f"waitsplit-{_SPLIT_SEQ[0]}",
                            "opcode": "NoOp",
                            "sync_info": {"on_update": [], "on_wait": [w]},
                        })
                    si["on_wait"] = [waits[-1]]
                out.append(inst)
            bb["instructions"] = out
    return m


_FIXED = [False]


def _install_fixups():
    if _FIXED[0]:
        return
    _FIXED[0] = True
    orig = bass.Bass.to_json_bytes

    def patched(self, *a, **k):
        return json.dumps(
            _split_multi_waits_json(json.loads(orig(self, *a, **k)))
        ).encode()

    bass.Bass.to_json_bytes = patched


# ---------------------------------------------------------------------------
# problem constants (hardcoded shapes)
# ---------------------------------------------------------------------------
N, E, D, H, O = 50000, 1_600_000, 128, 128, 64
NC, NBLK = 8, 49
NOWN = NBLK * 128           # 6272
NPAD = NC * NOWN            # 50176


def _host_prep(x, edge_index, w, T_b=None):
    src, dst = np.asarray(edge_index[0]), np.asarray(edge_index[1])
    x = np.asarray(x, np.float32)
    xpad = np.zeros((NPAD, D), np.float32)
    xpad[:N] = x
    xT = np.ascontiguousarray(xpad.T)

    per_core_blocks = []
    for c in range(NC):
        m = (dst // NOWN) == c
        s_c, d_c = src[m], dst[m] - c * NOWN
        order = np.argsort(d_c, kind="stable")
        s_c, d_c = s_c[order], d_c[order]
        bounds = np.searchsorted(d_c, np.arange(0, NOWN + 1, 128))
        per_core_blocks.append([
            (s_c[bounds[b] : bounds[b + 1]], d_c[bounds[b] : bounds[b + 1]] - b * 128)
            for b in range(NBLK)
        ])
    need = max(
        (len(s) + 127) // 128 for blocks in per_core_blocks for s, _ in blocks
    )
    if T_b is None:
        T_b = need
    assert need <= T_b
    TW = T_b * 128

    c1 = (w["bl1"] + w["br1"]).astype(np.float32)
    s1 = (w["blin1"] + w["bias1"] + w["bl1"]).astype(np.float32)
    c2 = (w["bl2"] + w["br2"]).astype(np.float32)
    s2 = (w["blin2"] + w["bias2"] + w["bl2"]).astype(np.float32)
    shared = {
        "xT": xT,
        "Wl1": np.asarray(w["Wl1"], np.float32), "Wr1": np.asarray(w["Wr1"], np.float32),
        "Wlin1": np.asarray(w["Wlin1"], np.float32),
        "Wl2": np.asarray(w["Wl2"], np.float32), "Wr2": np.asarray(w["Wr2"], np.float32),
        "Wlin2": np.asarray(w["Wlin2"], np.float32),
        "c1_rep": np.tile(c1, (128, 1)), "s1_rep": np.tile(s1, (128, 1)),
        "att1_rep": np.tile(np.asarray(w["att1"], ml_dtypes.bfloat16), (128, 4)),
        "c2_rep": np.tile(c2, (128, 1)), "s2_rep": np.tile(s2, (128, 1)),
        "att2_rep": np.tile(np.asarray(w["att2"], ml_dtypes.bfloat16), (128, 4)),
        "iota_col": np.arange(128, dtype=np.float32)[:, None],
        "iota_row": np.tile(np.arange(128, dtype=ml_dtypes.bfloat16), (128, 1)),
    }
    in_maps = []
    for c in range(NC):
        srcidx = np.zeros((NBLK, 128, T_b), np.int32)
        drel_row = np.full((NBLK, 1, TW), -1.0, np.float32)
        drel_col = np.full((NBLK, 128, T_b), -1.0, np.float32)
        for b, (s_b, drel_b) in enumerate(per_core_blocks[c]):
            n = len(s_b)
            sp = np.zeros(TW, np.int32)
            dp = np.full(TW, -1.0, np.float32)
            sp[:n] = s_b
            dp[:n] = drel_b
            srcidx[b] = sp.reshape(T_b, 128).T
            drel_col[b] = dp.reshape(T_b, 128).T
            drel_row[b, 0] = dp
        im = dict(shared)
        im["x_ownT"] = np.ascontiguousarray(xpad[c * NOWN : (c + 1) * NOWN].T)
        im["srcidx"] = srcidx
        im["drel_row"] = drel_row.astype(ml_dtypes.bfloat16)
        im["drel_col"] = drel_col
        in_maps.append(im)
    return in_maps, T_b


def _build_program(T_b):
    TW = T_b * 128
    NBT = NPAD // 128
    nc = bass.Bass()

    def din(name, shape, dt=F32):
        return nc.dram_tensor(name, shape, dt, kind="ExternalInput")

    xT = din("xT", [D, NPAD])
    x_ownT = din("x_ownT", [D, NOWN])
    srcidx = din("srcidx", [NBLK, 128, T_b], I32)
    drel_row = din("drel_row", [NBLK, 1, TW], BF16)
    drel_col = din("drel_col", [NBLK, 128, T_b])
    Wl1 = din("Wl1", [D, H]); Wr1 = din("Wr1", [D, H]); Wlin1 = din("Wlin1", [D, H])
    Wl2 = din("Wl2", [H, O]); Wr2 = din("Wr2", [H, O]); Wlin2 = din("Wlin2", [H, O])
    c1_rep = din("c1_rep", [128, H]); s1_rep = din("s1_rep", [128, H])
    att1_rep = din("att1_rep", [128, 4 * H], BF16)
    c2_rep = din("c2_rep", [128, O]); s2_rep = din("s2_rep", [128, O])
    att2_rep = din("att2_rep", [128, 4 * O], BF16)
    iota_col = din("iota_col", [128, 1])
    iota_row = din("iota_row", [128, 128], BF16)
    out_own = nc.dram_tensor("out_own", [NOWN, O], F32, kind="ExternalOutput")

    with tile.TileContext(nc) as tc:
        with (
            tc.tile_pool(name="dram", bufs=1, space="DRAM") as dram,
            tc.tile_pool(name="const", bufs=1) as cpool,
            tc.tile_pool(name="res", bufs=1) as rpool,
            tc.tile_pool(name="blk", bufs=2) as bpool,
            tc.tile_pool(name="oh", bufs=1) as ohpool,
            tc.tile_pool(name="g", bufs=3) as gpool,
            tc.tile_pool(name="ew", bufs=3) as epool,
            tc.tile_pool(name="sm", bufs=6) as smpool,
            tc.tile_pool(name="pu", bufs=2, space="PSUM") as pu,
            tc.tile_pool(name="px", bufs=2, space="PSUM") as px,
            tc.tile_pool(name="pr", bufs=2, space="PSUM") as pr,
            tc.tile_pool(name="pa", bufs=2, space="PSUM") as pa,
        ):
            xl1_full = dram.tile([NPAD, H], BF16)
            xl2_own_d = dram.tile([NOWN, O], BF16)
            xl2_full = dram.tile([NPAD, O], BF16)

            def ld(shape, apsrc, name, dt=F32):
                t = cpool.tile(shape, dt, tag=name)
                nc.sync.dma_start(out=t[:], in_=apsrc)
                return t

            Wl1_s = ld([D, H], Wl1[:], "Wl1")
            Wr1_s = ld([D, H], Wr1[:], "Wr1")
            Wlin1_s = ld([D, H], Wlin1[:], "Wlin1")
            Wl2_s = ld([H, O], Wl2[:], "Wl2")
            Wr2_s = ld([H, O], Wr2[:], "Wr2")
            Wlin2_s = ld([H, O], Wlin2[:], "Wlin2")
            c1_s = ld([128, H], c1_rep[:], "c1")
            s1_s = ld([128, H], s1_rep[:], "s1")
            a1_s = ld([128, 4 * H], att1_rep[:], "a1", BF16)
            c2_s = ld([128, O], c2_rep[:], "c2")
            s2_s = ld([128, O], s2_rep[:], "s2")
            a2_s = ld([128, 4 * O], att2_rep[:], "a2", BF16)
            ic_s = ld([128, 1], iota_col[:], "ic")
            ir_s = ld([128, 128], iota_row[:], "ir", BF16)
            ones1 = cpool.tile([1, 128], BF16, tag="ones1")
            nc.vector.memset(ones1[:], 1.0)
            ident = cpool.tile([128, 128], F32, tag="ident")
            make_identity(nc, ident[:])

            for blk in range(NBT):
                xtb = bpool.tile([D, 128], F32, tag="xtb")
                nc.sync.dma_start(out=xtb[:], in_=xT[:, blk * 128 : (blk + 1) * 128])
                ps = pa.tile([128, H], F32, tag="pa")
                nc.tensor.matmul(out=ps[:], lhsT=xtb[:], rhs=Wl1_s[:], start=True, stop=True)
                xlb = bpool.tile([128, H], BF16, tag="xlb")
                nc.vector.tensor_copy(out=xlb[:], in_=ps[:])
                nc.sync.dma_start(out=xl1_full[blk * 128 : (blk + 1) * 128, :], in_=xlb[:])

            xr_all = rpool.tile([128, NBLK * H], BF16, tag="xr_all")
            skip_all = rpool.tile([128, NBLK * H], F32, tag="skip_all")
            hT_all = rpool.tile([128, NBLK * 128], F32, tag="hT_all")
            for b in range(NBLK):
                xob = bpool.tile([D, 128], F32, tag="xob")
                nc.sync.dma_start(out=xob[:], in_=x_ownT[:, b * 128 : (b + 1) * 128])
                ps = pa.tile([128, H], F32, tag="pa")
                nc.tensor.matmul(out=ps[:], lhsT=xob[:], rhs=Wr1_s[:], start=True, stop=True)
                nc.vector.tensor_add(out=xr_all[:, b * H : (b + 1) * H], in0=ps[:], in1=c1_s[:])
                ps2 = pa.tile([128, H], F32, tag="pa")
                nc.tensor.matmul(out=ps2[:], lhsT=xob[:], rhs=Wlin1_s[:], start=True, stop=True)
                nc.vector.tensor_add(out=skip_all[:, b * H : (b + 1) * H], in0=ps2[:], in1=s1_s[:])

            xr2_all = rpool.tile([128, NBLK * O], BF16, tag="xr2_all")
            skip2_all = rpool.tile([128, NBLK * O], F32, tag="skip2_all")

            def edge_pass(layer):
                F = H if layer == 1 else O
                att_s = a1_s if layer == 1 else a2_s
                xr_src = xr_all if layer == 1 else xr2_all
                skip_src = skip_all if layer == 1 else skip2_all
                table = xl1_full if layer == 1 else xl2_full
                FW = F + 4  # g row stride (gather F cols + ones col + pad)
                for b in range(NBLK):
                    drow = bpool.tile([1, TW], BF16, tag="drow")
                    nc.sync.dma_start(out=drow[:], in_=drel_row[b, :, :])
                    idxb = bpool.tile([128, T_b], I32, tag="idxb")
                    nc.sync.dma_start(out=idxb[:], in_=srcidx[b, :, :])
                    dcol = bpool.tile([128, T_b], F32, tag="dcol")
                    nc.sync.dma_start(out=dcol[:], in_=drel_col[b, :, :])
                    oh = ohpool.tile([128, TW], BF16, tag="oh")
                    for c0 in range(0, TW, 512):
                        cw = min(512, TW - c0)
                        psr = pr.tile([128, 512], F32, tag="pr")
                        nc.tensor.matmul(
                            out=psr[:, :cw], lhsT=ones1[:],
                            rhs=drow[:, c0 : c0 + cw], start=True, stop=True,
                        )
                        nc.vector.tensor_scalar(
                            out=oh[:, c0 : c0 + cw], in0=psr[:, :cw],
                            scalar1=ic_s[:, :1], scalar2=None, op0=AL.is_equal,
                        )
                    gall = gpool.tile([128, T_b * FW], BF16, tag="g")
                    nc.vector.memset(
                        gall[:].rearrange("p (t f) -> p t f", f=FW)[:, :, F : F + 1],
                        1.0,
                    )
                    eall = smpool.tile([128, T_b], F32, tag="e")
                    for i0 in range(0, T_b, 4):
                        q = min(4, T_b - i0)
                        for i in range(i0, i0 + q):
                            nc.gpsimd.indirect_dma_start(
                                out=gall[:, i * FW : i * FW + F], out_offset=None,
                                in_=table[:],
                                in_offset=bass.IndirectOffsetOnAxis(
                                    ap=idxb[:, i : i + 1], axis=0
                                ),
                            )
                        psx = px.tile([128, 4 * F], F32, tag="px")
                        for i in range(i0, i0 + q):
                            nc.tensor.matmul(
                                out=psx[:, (i - i0) * F : (i - i0 + 1) * F],
                                lhsT=oh[:, i * 128 : (i + 1) * 128],
                                rhs=xr_src[:, b * F : (b + 1) * F],
                                start=True, stop=True,
                            )
                        g4 = gall[:].rearrange("p (t f) -> p t f", f=FW)[
                            :, i0 : i0 + q, :F
                        ]
                        su = epool.tile([128, 4 * F], BF16, tag="su")
                        nc.vector.tensor_add(
                            out=su[:, : q * F].rearrange("p (t f) -> p t f", f=F),
                            in0=g4, in1=psx[:, : q * F].rearrange("p (t f) -> p t f", f=F),
                        )
                        z = epool.tile([128, 4 * F], BF16, tag="z")
                        nc.vector.scalar_tensor_tensor(
                            out=z[:, : q * F], in0=su[:, : q * F], scalar=NEG,
                            in1=su[:, : q * F], op0=AL.mult, op1=AL.max,
                        )
                        wt = epool.tile([128, 4 * F], BF16, tag="wt")
                        nc.vector.tensor_mul(
                            out=wt[:, : q * F], in0=z[:, : q * F], in1=att_s[:, : q * F]
                        )
                        nc.vector.tensor_reduce(
                            out=eall[:, i0 : i0 + q],
                            in_=wt[:, : q * F].rearrange("p (t f) -> p t f", f=F),
                            axis=mybir.AxisListType.X, op=AL.add,
                        )
                    pall = smpool.tile([128, T_b], F32, tag="p")
                    nc.scalar.activation(out=pall[:], in_=eall[:], func=ACTF.Exp)
                    U = pu.tile([128, F + 1], F32, tag="pu")
                    for i in range(T_b):
                        seg = epool.tile([128, 128], BF16, tag="seg")
                        nc.vector.tensor_scalar(
                            out=seg[:], in0=ir_s[:], scalar1=dcol[:, i : i + 1],
                            scalar2=pall[:, i : i + 1], op0=AL.is_equal, op1=AL.mult,
                        )
                        nc.tensor.matmul(
                            out=U[:], lhsT=seg[:], rhs=gall[:, i * FW : i * FW + F + 1],
                            start=(i == 0), stop=(i == T_b - 1),
                        )
                    se = smpool.tile([128, 1], F32, tag="se")
                    nc.vector.tensor_scalar(
                        out=se[:], in0=U[:, F : F + 1], scalar1=1e-30,
                        scalar2=None, op0=AL.add,
                    )
                    r = smpool.tile([128, 1], F32, tag="r")
                    nc.vector.reciprocal(out=r[:], in_=se[:])
                    t1 = epool.tile([128, F], F32, tag="t1")
                    nc.vector.tensor_scalar(
                        out=t1[:], in0=U[:, :F], scalar1=r[:, :1],
                        scalar2=None, op0=AL.mult,
                    )
                    t2 = epool.tile([128, F], F32, tag="t2")
                    nc.vector.tensor_add(
                        out=t2[:], in0=t1[:], in1=skip_src[:, b * F : (b + 1) * F]
                    )
                    if layer == 1:
                        hb = epool.tile([128, F], F32, tag="hb")
                        nc.scalar.activation(out=hb[:], in_=t2[:], func=ACTF.Relu)
                        pst = pa.tile([128, 128], F32, tag="pa")
                        nc.tensor.transpose(out=pst[:], in_=hb[:], identity=ident[:])
                        nc.vector.tensor_copy(
                            out=hT_all[:, b * 128 : (b + 1) * 128], in_=pst[:]
                        )
                    else:
                        ob = epool.tile([128, F], F32, tag="ob")
                        nc.vector.tensor_copy(out=ob[:], in_=t2[:])
                        nc.sync.dma_start(
                            out=out_own[b * 128 : (b + 1) * 128, :], in_=ob[:]
                        )

            edge_pass(1)

            for b in range(NBLK):
                hTb = hT_all[:, b * 128 : (b + 1) * 128]
                ps = pa.tile([128, O], F32, tag="pa")
                nc.tensor.matmul(out=ps[:], lhsT=hTb, rhs=Wl2_s[:], start=True, stop=True)
                xl2b = bpool.tile([128, O], BF16, tag="xl2b")
                nc.vector.tensor_copy(out=xl2b[:], in_=ps[:])
                nc.sync.dma_start(out=xl2_own_d[b * 128 : (b + 1) * 128, :], in_=xl2b[:])
                ps2 = pa.tile([128, O], F32, tag="pa")
                nc.tensor.matmul(out=ps2[:], lhsT=hTb, rhs=Wr2_s[:], start=True, stop=True)
                nc.vector.tensor_add(out=xr2_all[:, b * O : (b + 1) * O], in0=ps2[:], in1=c2_s[:])
                ps3 = pa.tile([128, O], F32, tag="pa")
                nc.tensor.matmul(out=ps3[:], lhsT=hTb, rhs=Wlin2_s[:], start=True, stop=True)
                nc.vector.tensor_add(out=skip2_all[:, b * O : (b + 1) * O], in0=ps3[:], in1=s2_s[:])
            nc.gpsimd.collective_compute(
                "AllGather", AL.bypass,
                replica_groups=[list(range(NC))],
                ins=[xl2_own_d[:].opt()],
                outs=[xl2_full[:].opt()],
            )

            edge_pass(2)

    return nc


_W_KEYS = [
    "Wl1", "bl1", "Wr1", "br1", "att1", "bias1", "Wlin1", "blin1",
    "Wl2", "bl2", "Wr2", "br2", "att2", "bias2", "Wlin2", "blin2",
]


def kernel(x, edge_index, **w):
    _install_fixups()
    from concourse.bass_utils import run_bass_kernel_spmd

    w = {k: np.asarray(w[k], np.float32) for k in _W_KEYS}
    in_maps, T_b = _host_prep(np.asarray(x), np.asarray(edge_index), w)
    nc = _build_program(T_b)
    last_err = None
    for attempt in range(3):
        try:
            res = run_bass_kernel_spmd(nc, in_maps, core_ids=list(range(NC)))
            break
        except Exception as exc:  # flaky device recovery
            last_err = exc
            print(f"kernel: attempt {attempt} failed: {exc}", file=sys.stderr)
    else:
        raise last_err
    out = np.concatenate(
        [res.results[c]["out_own"] for c in range(NC)], axis=0
    )[:N]
    return out.astype(np.float32)

